# revision 5
# baseline (speedup 1.0000x reference)
"""Trainium2 Bass kernel for nn_GBGraphConvModel (DeepChem-style GraphConv).

8-way data-parallel plan:
  - conv1/pool1/conv2 run on 1/8 degree-sliced shards of the atom set;
    the full per-atom activation arrays (h1, p1, h2) are exchanged with
    on-chip AllGather collectives between stages.
  - pool2 + dense + readout rows are sharded by MOLECULE, so the
    segment_sum/segment_max readout is purely core-local (matches the
    intended "data-parallel over molecules" sharding for the output).
  - gathers use GPSIMD indirect DMA (one 256/300B descriptor per row).

kernel(**inputs) takes the FULL inputs and returns the FULL [4096, 1]
output; all sharding happens inside.
"""
import contextlib

import numpy as np

import concourse.bass as bass
import concourse.mybir as mybir
import concourse.tile as tile
from concourse import bacc
from concourse.bass_utils import run_bass_kernel_spmd

# ---- static problem shape ----
COUNTS = [1024, 24576, 36864, 36864, 20480, 2048, 512, 256, 128, 64, 64]
OFFS = np.cumsum([0] + COUNTS)
N_ATOMS = int(OFFS[-1])           # 122880
BATCH = 4096
F_IN = 75
HID = 64
MAXDEG = 10
EPS = 1e-3
NCORE = 8
P = 128

FP = mybir.dt.float32
I32 = mybir.dt.int32


def _pad128(n):
    return (int(n) + P - 1) // P * P


SH_D = [COUNTS[d] // NCORE for d in range(MAXDEG + 1)]   # per-core class size
SH_DP = [_pad128(s) for s in SH_D]
SH_OFF = np.cumsum([0] + SH_DP)
SH_ROWS = int(SH_OFF[-1])
AG_ROWS = SH_ROWS * NCORE

_SH_D_ARR = np.array(SH_D)
_SH_OFF_ARR = np.array(SH_OFF[:-1])


def _global_to_padded(g):
    """global atom id -> row in the padded AllGather activation layout."""
    g = np.asarray(g, np.int64)
    d = np.searchsorted(OFFS, g, side="right") - 1
    p = g - OFFS[d]
    n8 = _SH_D_ARR[d]
    core = p // n8
    loc = p % n8
    return (core * SH_ROWS + _SH_OFF_ARR[d] + loc).astype(np.int32)


def _op_major(cols):
    """[n] index list -> [128, m] tile-op-major layout (op j = col j)."""
    n = len(cols)
    m = n // P
    return np.ascontiguousarray(cols.reshape(m, P).T).astype(np.int32)


class Plan:
    """Host-side index preprocessing (pure rearrangement of input indices)."""

    def __init__(self, membership, adjs):
        self.idx_atoms = {}   # (d, k) -> [NCORE][128, m] global atom ids
        self.idx_pad = {}     # (d, k) -> [NCORE][128, m] padded AG ids
        for d in range(1, MAXDEG + 1):
            adj = np.asarray(adjs[d - 1])
            n8 = SH_D[d]
            for k in range(d):
                ga, pa = [], []
                for c in range(NCORE):
                    sl = np.zeros(SH_DP[d], np.int64)
                    sl[:n8] = adj[c * n8:(c + 1) * n8, k]
                    ga.append(_op_major(sl.astype(np.int32)))
                    pa.append(_op_major(_global_to_padded(sl)))
                self.idx_atoms[(d, k)] = ga
                self.idx_pad[(d, k)] = pa

        # conv1 self rows: per-core [SH_ROWS] global ids (padded tails -> 0)
        self.self_atom_rows = []
        for c in range(NCORE):
            rows = np.zeros(SH_ROWS, np.int64)
            for d in range(MAXDEG + 1):
                o = SH_OFF[d]
                n8 = SH_D[d]
                rows[o:o + n8] = OFFS[d] + c * n8 + np.arange(n8)
            self.self_atom_rows.append(rows)

        # ---- final stage (molecule-owned) ----
        mol_of = np.asarray(membership, np.int64)
        deg_of = np.searchsorted(OFFS, np.arange(N_ATOMS), side="right") - 1
        self.MOLS = BATCH // NCORE
        core_of = mol_of // self.MOLS
        lists = {}
        cnt = np.zeros((NCORE, MAXDEG + 1), np.int64)
        for c in range(NCORE):
            g = np.where(core_of == c)[0]
            dg = deg_of[g]
            for d in range(MAXDEG + 1):
                gl = np.sort(g[dg == d])
                lists[(c, d)] = gl
                cnt[c, d] = len(gl)
        self.F_DP = [_pad128(cnt[:, d].max()) for d in range(MAXDEG + 1)]
        self.F_OFF = np.cumsum([0] + self.F_DP)
        self.F_ROWS = int(self.F_OFF[-1])

        adj_by = {d: np.asarray(adjs[d - 1]) for d in range(1, MAXDEG + 1)}
        self.f_self = []          # [NCORE][128, F_ROWS//128] padded AG ids
        self.f_adj = {}           # (d, k) -> [NCORE][128, m]
        self.f_rows_g = np.zeros((NCORE, self.F_ROWS), np.int64)
        self.f_valid = np.zeros((NCORE, self.F_ROWS), bool)
        for c in range(NCORE):
            srows = np.zeros(self.F_ROWS, np.int64)
            for d in range(MAXDEG + 1):
                gl = lists[(c, d)]
                o = self.F_OFF[d]
                srows[o:o + len(gl)] = gl
                self.f_rows_g[c, o:o + len(gl)] = gl
                self.f_valid[c, o:o + len(gl)] = True
            self.f_self.append(_op_major(_global_to_padded(srows)))
        for d in range(1, MAXDEG + 1):
            for k in range(d):
                arrs = []
                for c in range(NCORE):
                    gl = lists[(c, d)]
                    a = np.zeros(self.F_DP[d], np.int64)
                    if len(gl):
                        a[:len(gl)] = adj_by[d][gl - OFFS[d], k]
                    arrs.append(_op_major(_global_to_padded(a)))
                self.f_adj[(d, k)] = arrs

        # ---- readout slot maps ----
        # z rows per core: F_ROWS real (+2 sentinel tiles appended by kernel:
        # rows [F_ROWS, F_ROWS+128) = -1e30 (max), [F_ROWS+128, ..+256) = 0.
        SENT_MAX = self.F_ROWS
        SENT_SUM = self.F_ROWS + P
        sizes = np.zeros((NCORE, self.MOLS), np.int64)
        atoms_of = {}
        asort = np.argsort(mol_of, kind="stable")
        msorted = mol_of[asort]
        for c in range(NCORE):
            zrow = np.full(N_ATOMS, -1, np.int64)
            vr = self.f_rows_g[c][self.f_valid[c]]
            zrow[vr] = np.where(self.f_valid[c])[0]
            base = c * self.MOLS
            for mi in range(self.MOLS):
                lo = np.searchsorted(msorted, base + mi, side="left")
                hi = np.searchsorted(msorted, base + mi, side="right")
                a = zrow[asort[lo:hi]]
                atoms_of[(c, mi)] = a[a >= 0]
                sizes[c, mi] = len(atoms_of[(c, mi)])
        self.mol_order = np.argsort(-sizes, axis=1, kind="stable")
        max_slots = int(sizes.max())
        # per-slot mol coverage (max over cores -> SPMD-uniform tiling)
        slot_mols = np.zeros(max_slots, np.int64)
        for s in range(max_slots):
            slot_mols[s] = _pad128(max(1, int((sizes > s).sum(axis=1).max())))
        slot_mols[0] = self.MOLS   # every molecule covered by slot 0
        self.slot_mols = slot_mols                    # padded mols per slot
        # build the concatenated slot index stream per core (both maps)
        self.slot_stream_max = []
        self.slot_stream_sum = []
        # per-slot tile structure: [(slot, tile_idx_in_slot, mol_tile)] with
        # mol_tile = which acc tile (0..MOLS_P/128) it accumulates into
        self.slot_tiles = []
        for s in range(max_slots):
            for t in range(slot_mols[s] // P):
                self.slot_tiles.append((s, t))
        def wrap16(flat):
            # dma_gather index layout: idx j at [j % 16, j // 16], the
            # 16-partition block replicated for each of the 8 Q7 cores.
            w = np.zeros((16, len(flat) // 16), np.int16)
            w[np.arange(len(flat)) % 16, np.arange(len(flat)) // 16] = flat
            return np.tile(w, (8, 1))

        for c in range(NCORE):
            sm, ss = [], []
            for s in range(max_slots):
                n = slot_mols[s]
                a_max = np.full(n, SENT_MAX, np.int64)
                a_sum = np.full(n, SENT_SUM, np.int64)
                for pos in range(min(n, self.MOLS)):
                    mi = self.mol_order[c, pos]
                    if sizes[c, mi] > s:
                        a_max[pos] = atoms_of[(c, mi)][s]
                        a_sum[pos] = atoms_of[(c, mi)][s]
                sm.append(a_max)
                ss.append(a_sum)
            self.slot_stream_max.append(wrap16(np.concatenate(sm)))
            self.slot_stream_sum.append(wrap16(np.concatenate(ss)))
        self.n_slot_rows = int(sum(slot_mols))


def build_kernel(plan):
    nc = bacc.Bacc("TRN2", target_bir_lowering=False, debug=False,
                   num_swdge_queues=4)
    MOLS = plan.MOLS          # 512
    MT = MOLS // P            # 4 mol tiles

    # ---------------- I/O ----------------
    atoms = nc.dram_tensor("atoms", [N_ATOMS, F_IN], FP, kind="ExternalInput")
    atoms_self = nc.dram_tensor("atoms_self", [SH_ROWS, F_IN], FP,
                                kind="ExternalInput")
    atoms_rel = nc.dram_tensor("atoms_rel", [SH_ROWS, F_IN], FP,
                               kind="ExternalInput")
    atoms_relT = nc.dram_tensor("atoms_relT", [F_IN, SH_ROWS], FP,
                                kind="ExternalInput")
    atoms_selfT = nc.dram_tensor("atoms_selfT", [F_IN, SH_ROWS], FP,
                                 kind="ExternalInput")
    idxA, idxP1, idxF = {}, {}, {}
    for d in range(1, MAXDEG + 1):
        m = SH_DP[d] // P
        for k in range(d):
            idxA[(d, k)] = nc.dram_tensor(f"ia_{d}_{k}", [P, m], I32,
                                          kind="ExternalInput")
            idxP1[(d, k)] = nc.dram_tensor(f"ip_{d}_{k}", [P, m], I32,
                                           kind="ExternalInput")
        mf = plan.F_DP[d] // P
        for k in range(d):
            idxF[(d, k)] = nc.dram_tensor(f"if_{d}_{k}", [P, mf], I32,
                                          kind="ExternalInput")
    fself = nc.dram_tensor("fself", [P, plan.F_ROWS // P], I32,
                           kind="ExternalInput")
    n_slot_tiles = len(plan.slot_tiles)
    n_slot_rows = n_slot_tiles * P
    slotmax = nc.dram_tensor("slotmax", [P, n_slot_rows // 16], mybir.dt.int16,
                             kind="ExternalInput")
    slotsum = nc.dram_tensor("slotsum", [P, n_slot_rows // 16], mybir.dt.int16,
                             kind="ExternalInput")
    wc1 = nc.dram_tensor("wc1", [2 * MAXDEG + 1, F_IN, HID], FP, kind="ExternalInput")
    wc2 = nc.dram_tensor("wc2", [2 * MAXDEG + 1, HID, HID], FP, kind="ExternalInput")
    bc1r = nc.dram_tensor("bc1r", [P, (2 * MAXDEG + 1) * HID], FP, kind="ExternalInput")
    bc2r = nc.dram_tensor("bc2r", [P, (2 * MAXDEG + 1) * HID], FP, kind="ExternalInput")
    bn1sr = nc.dram_tensor("bn1sr", [P, HID], FP, kind="ExternalInput")
    bn1br = nc.dram_tensor("bn1br", [P, HID], FP, kind="ExternalInput")
    bn3sr = nc.dram_tensor("bn3sr", [P, 128], FP, kind="ExternalInput")
    bn3br = nc.dram_tensor("bn3br", [P, 128], FP, kind="ExternalInput")
    d1wt = nc.dram_tensor("d1wt", [HID, 128], FP, kind="ExternalInput")
    d1br = nc.dram_tensor("d1br", [P, 128], FP, kind="ExternalInput")
    d2wf = nc.dram_tensor("d2wf", [P, 2], FP, kind="ExternalInput")  # prescaled
    d3wr = nc.dram_tensor("d3wr", [15, 1], FP, kind="ExternalInput")
    cbias = nc.dram_tensor("cbias", [P, 1], FP, kind="ExternalInput")
    npad2 = nc.dram_tensor("npad2", [P, 4], FP, kind="ExternalInput")
    xaddT = nc.dram_tensor("xaddT", [15, MOLS], FP, kind="ExternalInput")
    ident = nc.dram_tensor("identity", [P, P], FP, kind="ExternalInput")
    out = nc.dram_tensor("out", [MOLS, 1], FP, kind="ExternalOutput")

    # ---------------- internal DRAM ----------------
    h1_sh = nc.dram_tensor("h1_sh", [SH_ROWS, HID], FP)
    p1_sh = nc.dram_tensor("p1_sh", [SH_ROWS, HID], FP)
    h2_sh = nc.dram_tensor("h2_sh", [SH_ROWS, HID], FP)
    h1 = nc.dram_tensor("h1", [AG_ROWS, HID], FP, addr_space="Shared")
    p1 = nc.dram_tensor("p1", [AG_ROWS, HID], FP, addr_space="Shared")
    h2 = nc.dram_tensor("h2", [AG_ROWS, HID], FP, addr_space="Shared")
    p2b = nc.dram_tensor("p2b", [plan.F_ROWS, HID], FP)
    zbuf = nc.dram_tensor("zbuf", [plan.F_ROWS + 2 * P, 128], FP)

    with tile.TileContext(nc) as tc:
        with contextlib.ExitStack() as ctx:
            cpool = ctx.enter_context(tc.tile_pool(name="consts", bufs=1))
            gpool = ctx.enter_context(tc.tile_pool(name="gather", bufs=12))
            ipool = ctx.enter_context(tc.tile_pool(name="idx", bufs=14))
            tpool = ctx.enter_context(tc.tile_pool(name="tmp", bufs=8))
            pst = ctx.enter_context(tc.tile_pool(name="pst", bufs=2, space="PSUM"))
            psm = ctx.enter_context(tc.tile_pool(name="psm", bufs=2, space="PSUM"))

            idt = cpool.tile([P, P], FP)
            nc.sync.dma_start(out=idt[:], in_=ident[:])
            b1s = cpool.tile([P, HID], FP)
            b1b = cpool.tile([P, HID], FP)
            b3s = cpool.tile([P, 128], FP)
            b3b = cpool.tile([P, 128], FP)
            db1 = cpool.tile([P, 128], FP)
            nc.sync.dma_start(out=b1s[:], in_=bn1sr[:])
            nc.sync.dma_start(out=b1b[:], in_=bn1br[:])
            nc.sync.dma_start(out=b3s[:], in_=bn3sr[:])
            nc.sync.dma_start(out=b3b[:], in_=bn3br[:])
            nc.sync.dma_start(out=db1[:], in_=d1br[:])
            w1t = cpool.tile([F_IN, (2 * MAXDEG + 1) * HID], FP)
            w2t = cpool.tile([HID, (2 * MAXDEG + 1) * HID], FP)
            bb1 = cpool.tile([P, (2 * MAXDEG + 1) * HID], FP)
            bb2 = cpool.tile([P, (2 * MAXDEG + 1) * HID], FP)
            nc.sync.dma_start(out=w1t[:].rearrange("k (w h) -> k w h", w=2 * MAXDEG + 1),
                              in_=wc1.rearrange("w k h -> k w h"))
            nc.sync.dma_start(out=w2t[:].rearrange("k (w h) -> k w h", w=2 * MAXDEG + 1),
                              in_=wc2.rearrange("w k h -> k w h"))
            nc.sync.dma_start(out=bb1[:], in_=bc1r[:])
            nc.sync.dma_start(out=bb2[:], in_=bc2r[:])
            d1w_t = cpool.tile([HID, 128], FP)
            nc.sync.dma_start(out=d1w_t[:], in_=d1wt[:])
            d2w_t = cpool.tile([P, 2], FP)
            nc.sync.dma_start(out=d2w_t[:], in_=d2wf[:])
            d3w_t = cpool.tile([15, 1], FP)
            nc.sync.dma_start(out=d3w_t[:], in_=d3wr[:])
            cb_t = cpool.tile([P, 1], FP)
            nc.sync.dma_start(out=cb_t[:], in_=cbias[:])
            npd_t = cpool.tile([P, 4], FP)
            nc.sync.dma_start(out=npd_t[:], in_=npad2[:])

            def wslice(wt, widx):
                return wt[:, widx * HID:(widx + 1) * HID]

            def gather_tile(dst_ap, src_dram, idx_tile_col, accum=False):
                nc.gpsimd.indirect_dma_start(
                    out=dst_ap, out_offset=None, in_=src_dram[:],
                    in_offset=bass.IndirectOffsetOnAxis(ap=idx_tile_col, axis=0),
                    compute_op=(mybir.AluOpType.add if accum
                                else mybir.AluOpType.bypass))

            def conv_epilogue(ps_o, bt, bidxs, bn_s, bn_b, dst_slice, width):
                hsb = tpool.tile([P, width], FP, tag="h")
                nc.vector.tensor_tensor(
                    out=hsb[:], in0=ps_o,
                    in1=bt[:, bidxs[0] * width:(bidxs[0] + 1) * width],
                    op=mybir.AluOpType.add)
                if len(bidxs) > 1:
                    nc.vector.tensor_tensor(
                        out=hsb[:], in0=hsb[:],
                        in1=bt[:, bidxs[1] * width:(bidxs[1] + 1) * width],
                        op=mybir.AluOpType.add)
                nc.scalar.activation(hsb[:], hsb[:],
                                     mybir.ActivationFunctionType.Tanh)
                nc.vector.tensor_tensor(out=hsb[:], in0=hsb[:], in1=bn_s[:],
                                        op=mybir.AluOpType.mult)
                nc.vector.tensor_tensor(out=hsb[:], in0=hsb[:], in1=bn_b[:],
                                        op=mybir.AluOpType.add)
                nc.sync.dma_start(out=dst_slice, in_=hsb[:])

            def transpose_to(src_tile, rows, into_pool, tag):
                ps_full = pst.tile([P, P], FP, tag="tr", name="trps")
                ps = ps_full[0:rows, :]
                dstT = into_pool.tile([rows, P], FP, tag="T" + tag)
                nc.tensor.transpose(out=ps[:], in_=src_tile, identity=idt[:])
                nc.vector.tensor_copy(out=dstT[:], in_=ps[:])
                return dstT

            def conv_stage(src_dram, src_f, idx_map, self_reader, wt, bt, dst,
                           sizes, offs, rel_src=None, relT_src=None,
                           selfT_src=None):
                for d in range(MAXDEG + 1):
                    m = sizes[d] // P
                    its = {}
                    if rel_src is None:
                        for k in range(d):
                            it = ipool.tile([P, m], I32, tag="idx")
                            nc.sync.dma_start(out=it[:], in_=idx_map[(d, k)][:])
                            its[k] = it
                    for j in range(m):
                        o = offs[d] + j * P
                        if d >= 1:
                            if relT_src is not None:
                                relT = tpool.tile([src_f, P], FP, tag="Tr")
                                nc.sync.dma_start(out=relT[:],
                                                  in_=relT_src[:, o:o + P])
                            else:
                                rel = gpool.tile([P, src_f], FP, tag="rel")
                                if rel_src is not None:
                                    nc.sync.dma_start(out=rel[:],
                                                      in_=rel_src[o:o + P, :])
                                else:
                                    for k in range(d):
                                        gather_tile(rel[:], src_dram,
                                                    its[k][:, j:j + 1],
                                                    accum=(k > 0))
                                relT = transpose_to(rel[:], src_f, tpool, "r")
                        if selfT_src is not None:
                            slfT = tpool.tile([src_f, P], FP, tag="Ts")
                            nc.sync.dma_start(out=slfT[:],
                                              in_=selfT_src[:, o:o + P])
                        else:
                            slf = gpool.tile([P, src_f], FP, tag="slf")
                            self_reader(d, j, slf)
                            slfT = transpose_to(slf[:], src_f, tpool, "s")
                        ps_o = psm.tile([P, HID], FP, tag="mm")
                        if d >= 1:
                            nc.tensor.matmul(out=ps_o[:], lhsT=relT[:],
                                             rhs=wslice(wt, 2 * (d - 1)),
                                             start=True, stop=False)
                            nc.tensor.matmul(out=ps_o[:], lhsT=slfT[:],
                                             rhs=wslice(wt, 2 * d - 1),
                                             start=False, stop=True)
                            bidxs = [2 * (d - 1), 2 * d - 1]
                        else:
                            nc.tensor.matmul(out=ps_o[:], lhsT=slfT[:],
                                             rhs=wslice(wt, 2 * MAXDEG),
                                             start=True, stop=True)
                            bidxs = [2 * MAXDEG]
                        conv_epilogue(ps_o[:], bt, bidxs, b1s, b1b,
                                      dst[offs[d] + j * P:offs[d] + (j + 1) * P, :],
                                      HID)

            def pool_stage(src_dram, idx_map, self_reader, dst, sizes, offs):
                for d in range(MAXDEG + 1):
                    m = sizes[d] // P
                    its = {}
                    for k in range(d):
                        it = ipool.tile([P, m], I32, tag="idx")
                        nc.sync.dma_start(out=it[:], in_=idx_map[(d, k)][:])
                        its[k] = it
                    for j in range(m):
                        acc = gpool.tile([P, HID], FP, tag="pacc")
                        self_reader(d, j, acc)
                        for k in range(d):
                            g = gpool.tile([P, HID], FP, tag="pg")
                            gather_tile(g[:], src_dram, its[k][:, j:j + 1])
                            nc.vector.tensor_tensor(out=acc[:], in0=acc[:],
                                                    in1=g[:],
                                                    op=mybir.AluOpType.max)
                        nc.sync.dma_start(
                            out=dst[offs[d] + j * P:offs[d] + (j + 1) * P, :],
                            in_=acc[:])

            def allgather(src_sh, full):
                nc.gpsimd.collective_compute(
                    "AllGather", mybir.AluOpType.bypass,
                    replica_groups=[list(range(NCORE))],
                    ins=[src_sh[:]], outs=[full[:]])

            # ---- conv1 ----
            def self_conv1(d, j, dst_tile):
                o = SH_OFF[d] + j * P
                nc.sync.dma_start(out=dst_tile[:], in_=atoms_self[o:o + P, :])

            conv_stage(atoms, F_IN, idxA, self_conv1, w1t, bb1, h1_sh,
                       SH_DP, SH_OFF, relT_src=atoms_relT,
                       selfT_src=atoms_selfT)
            allgather(h1_sh, h1)

            # ---- pool1 ---- (self rows are the local shard block)
            def self_local(src):
                def rd(d, j, dst_tile):
                    o = SH_OFF[d] + j * P
                    nc.sync.dma_start(out=dst_tile[:], in_=src[o:o + P, :])
                return rd

            pool_stage(h1, idxP1, self_local(h1_sh), p1_sh, SH_DP, SH_OFF)
            allgather(p1_sh, p1)

            # ---- conv2 ----
            conv_stage(p1, HID, idxP1, self_local(p1_sh), w2t, bb2, h2_sh,
                       SH_DP, SH_OFF)
            allgather(h2_sh, h2)

            # ---- pool2 (molecule-owned rows) ----
            fs_t = ipool.tile([P, plan.F_ROWS // P], I32, tag="fself")
            nc.sync.dma_start(out=fs_t[:], in_=fself[:])

            def self_f(d, j, dst_tile):
                col = (plan.F_OFF[d] // P) + j
                gather_tile(dst_tile[:], h2, fs_t[:, col:col + 1])

            pool_stage(h2, idxF, self_f, p2b, plan.F_DP, plan.F_OFF)

            # ---- dense d1 + tanh + bn3 -> zbuf ----
            for j in range(plan.F_ROWS // P):
                pt = gpool.tile([P, HID], FP, tag="pacc")
                nc.sync.dma_start(out=pt[:], in_=p2b[j * P:(j + 1) * P, :])
                ptT = transpose_to(pt[:], HID, tpool, "z")
                ps_o = psm.tile([P, 128], FP, tag="mmz")
                nc.tensor.matmul(out=ps_o[:], lhsT=ptT[:], rhs=d1w_t[:],
                                 start=True, stop=True)
                zt = tpool.tile([P, 128], FP, tag="zt")
                nc.vector.tensor_tensor(out=zt[:], in0=ps_o[:], in1=db1[:],
                                        op=mybir.AluOpType.add)
                nc.scalar.activation(zt[:], zt[:],
                                     mybir.ActivationFunctionType.Tanh)
                nc.vector.tensor_tensor(out=zt[:], in0=zt[:], in1=b3s[:],
                                        op=mybir.AluOpType.mult)
                nc.vector.tensor_tensor(out=zt[:], in0=zt[:], in1=b3b[:],
                                        op=mybir.AluOpType.add)
                nc.sync.dma_start(out=zbuf[j * P:(j + 1) * P, :], in_=zt[:])
            # sentinel tiles
            sent = tpool.tile([P, 128], FP, tag="zt")
            nc.vector.memset(sent[:], -2.0)
            nc.sync.dma_start(out=zbuf[plan.F_ROWS:plan.F_ROWS + P, :], in_=sent[:])
            sent2 = tpool.tile([P, 128], FP, tag="zt")
            nc.vector.memset(sent2[:], 0.0)
            nc.sync.dma_start(out=zbuf[plan.F_ROWS + P:plan.F_ROWS + 2 * P, :],
                              in_=sent2[:])

            # ---- readout: slot-gather accumulate ----
            smx_t = ipool.tile([P, n_slot_rows // 16], mybir.dt.int16, tag="smx")
            nc.sync.dma_start(out=smx_t[:], in_=slotmax[:])
            accs = [cpool.tile([P, 128], FP, name=f"accs{t}") for t in range(MT)]
            accm = [cpool.tile([P, 128], FP, name=f"accm{t}") for t in range(MT)]
            first_s = [True] * MT
            first_m = [True] * MT
            CH_RO = 4096            # slot rows per dma_gather op
            TPC = CH_RO // P        # tiles per chunk
            qn = [0]
            for base in range(0, n_slot_tiles, TPC):
                ntile = min(TPC, n_slot_tiles - base)
                nidx = ntile * P
                gt = gpool.tile([P, TPC * 128], FP, tag="rog", bufs=2)
                nc.gpsimd.dma_gather(
                    out_ap=gt[:, :ntile * 128].rearrange(
                        "p (c f) -> p c f", c=ntile),
                    in_ap=zbuf[:],
                    idxs_ap=smx_t[:, (base * P) // 16:(base * P + nidx) // 16],
                    num_idxs=nidx, num_idxs_reg=nidx, elem_size=128,
                    single_packet=False, queue_num=qn[0] % 4,
                )
                qn[0] += 1
                for ci in range(ntile):
                    s, t = plan.slot_tiles[base + ci]
                    sl = gt[:, ci * 128:(ci + 1) * 128]
                    for acc, first, op in (
                            (accm, first_m, mybir.AluOpType.max),
                            (accs, first_s, mybir.AluOpType.add)):
                        if first[t]:
                            nc.vector.tensor_copy(out=acc[t][:], in_=sl)
                            first[t] = False
                        else:
                            nc.vector.tensor_tensor(out=acc[t][:],
                                                    in0=acc[t][:],
                                                    in1=sl, op=op)

            # ---- final: out = tanh(s|m) @ d2w' + xadd @ d3w[1:] + cb ----
            xa_t = cpool.tile([15, MOLS], FP)
            nc.sync.dma_start(out=xa_t[:], in_=xaddT[:])
            for t in range(MT):
                nc.vector.tensor_scalar(
                    out=accs[t][:], in0=accs[t][:],
                    scalar1=npd_t[:, t:t + 1], scalar2=None,
                    op0=mybir.AluOpType.add)
                nc.scalar.activation(accs[t][:], accs[t][:],
                                     mybir.ActivationFunctionType.Tanh)
                nc.scalar.activation(accm[t][:], accm[t][:],
                                     mybir.ActivationFunctionType.Tanh)
                sT = transpose_to(accs[t][:], P, tpool, "f")
                mT = transpose_to(accm[t][:], P, tpool, "f")
                ps_f = psm.tile([P, 1], FP, tag="fin")
                nc.tensor.matmul(out=ps_f[:], lhsT=sT[:], rhs=d2w_t[:, 0:1],
                                 start=True, stop=False)
                nc.tensor.matmul(out=ps_f[:], lhsT=mT[:], rhs=d2w_t[:, 1:2],
                                 start=False, stop=False)
                nc.tensor.matmul(out=ps_f[:], lhsT=xa_t[:, t * P:(t + 1) * P],
                                 rhs=d3w_t[:], start=False, stop=True)
                ot = tpool.tile([P, 1], FP, tag="ot")
                nc.vector.tensor_tensor(out=ot[:], in0=ps_f[:], in1=cb_t[:],
                                        op=mybir.AluOpType.add)
                nc.sync.dma_start(out=out[t * P:(t + 1) * P, :], in_=ot[:])

    nc.compile()
    return nc


_CACHE = {}


def kernel(**inputs):
    atoms = np.asarray(inputs["atoms"], np.float32)
    membership = np.asarray(inputs["membership"], np.int32)
    adjs = [np.asarray(inputs[f"adj{d}"], np.int32) for d in range(1, MAXDEG + 1)]

    key = "k"
    if key not in _CACHE:
        plan = Plan(membership, adjs)
        nc = build_kernel(plan)
        _CACHE[key] = (plan, nc)
    plan, nc = _CACHE[key]

    # ---- parameter preprocessing (host, O(param size)) ----
    gc1_W = np.asarray(inputs["gc1_W"], np.float32)
    gc1_b = np.asarray(inputs["gc1_b"], np.float32)
    gc2_W = np.asarray(inputs["gc2_W"], np.float32)
    gc2_b = np.asarray(inputs["gc2_b"], np.float32)
    bn1_s = (np.asarray(inputs["bn1_gamma"], np.float32)
             / np.sqrt(np.asarray(inputs["bn1_var"], np.float32) + EPS))
    bn1_b = (np.asarray(inputs["bn1_beta"], np.float32)
             - np.asarray(inputs["bn1_mean"], np.float32) * bn1_s)
    bn3_s = (np.asarray(inputs["bn3_gamma"], np.float32)
             / np.sqrt(np.asarray(inputs["bn3_var"], np.float32) + EPS))
    bn3_b = (np.asarray(inputs["bn3_beta"], np.float32)
             - np.asarray(inputs["bn3_mean"], np.float32) * bn3_s)
    d1_W = np.asarray(inputs["d1_W"], np.float32)
    d1_b = np.asarray(inputs["d1_b"], np.float32)
    d2_W = np.asarray(inputs["d2_W"], np.float32)
    d2_b = np.asarray(inputs["d2_b"], np.float32)
    d3_W = np.asarray(inputs["d3_W"], np.float32)
    d3_b = np.asarray(inputs["d3_b"], np.float32)
    x_add = np.asarray(inputs["x_add"], np.float32)

    d2w_scaled = d2_W * d3_W[0, 0]                       # [256, 1]
    cb = float(d2_b[0] * d3_W[0, 0] + d3_b[0])

    def rep(v, w):
        return np.tile(np.asarray(v, np.float32)[None, :], (P, 1)).reshape(P, w)

    common = {
        "atoms": atoms,
        "wc1": gc1_W, "wc2": gc2_W,
        "bc1r": np.tile(gc1_b.reshape(1, -1), (P, 1)),
        "bc2r": np.tile(gc2_b.reshape(1, -1), (P, 1)),
        "bn1sr": rep(bn1_s, HID), "bn1br": rep(bn1_b, HID),
        "bn3sr": rep(bn3_s, 128), "bn3br": rep(bn3_b, 128),
        "d1wt": d1_W, "d1br": rep(d1_b, 128),
        "d2wf": np.ascontiguousarray(d2w_scaled.reshape(2, P).T),
        "d3wr": d3_W[1:16],
        "cbias": np.full((P, 1), cb, np.float32),
        "identity": np.eye(P, dtype=np.float32),
    }
    in_maps = []
    for c in range(NCORE):
        m = dict(common)
        m["atoms_self"] = np.ascontiguousarray(atoms[plan.self_atom_rows[c]])
        rel1 = np.zeros((SH_ROWS, F_IN), np.float32)
        for d in range(1, MAXDEG + 1):
            n8 = SH_D[d]
            blk = atoms[adjs[d - 1][c * n8:(c + 1) * n8]].sum(axis=1)
            rel1[SH_OFF[d]:SH_OFF[d] + n8] = blk
        m["atoms_rel"] = rel1
        m["atoms_relT"] = np.ascontiguousarray(rel1.T)
        m["atoms_selfT"] = np.ascontiguousarray(
            atoms[plan.self_atom_rows[c]].T)
        for d in range(1, MAXDEG + 1):
            for k in range(d):
                m[f"ia_{d}_{k}"] = plan.idx_atoms[(d, k)][c]
                m[f"ip_{d}_{k}"] = plan.idx_pad[(d, k)][c]
                m[f"if_{d}_{k}"] = plan.f_adj[(d, k)][c]
        m["fself"] = plan.f_self[c]
        m["slotmax"] = plan.slot_stream_max[c]
        m["slotsum"] = plan.slot_stream_sum[c]
        cover = np.array([(plan.slot_mols > pos).sum()
                          for pos in range(plan.MOLS)], np.int64)
        counts = np.bincount(np.asarray(membership, np.int64),
                             minlength=BATCH)
        sz = counts[c * plan.MOLS + plan.mol_order[c]]
        npad_pos = cover - sz
        assert npad_pos.min() >= 0
        m["npad2"] = np.ascontiguousarray(
            (2.0 * npad_pos.reshape(4, P).T).astype(np.float32))
        xa = x_add[c * plan.MOLS:(c + 1) * plan.MOLS][plan.mol_order[c]]
        m["xaddT"] = np.ascontiguousarray(xa.T)          # [15, MOLS]
        in_maps.append(m)

    res = run_bass_kernel_spmd(nc, in_maps, core_ids=list(range(NCORE)))
    kernel._last_results = res

    outp = np.zeros((BATCH, 1), np.float32)
    for c in range(NCORE):
        o = np.asarray(res.results[c]["out"])            # sorted-mol order
        inv = np.empty(plan.MOLS, np.int64)
        inv[plan.mol_order[c]] = np.arange(plan.MOLS)
        outp[c * plan.MOLS:(c + 1) * plan.MOLS] = o[inv]
    return outp



# revision 8
# speedup vs baseline: 1.0697x; 1.0697x over previous
"""Trainium2 Bass kernel for nn_GBGraphConvModel (DeepChem-style GraphConv).

8-way data-parallel plan:
  - conv1/pool1/conv2 run on 1/8 degree-sliced shards of the atom set;
    the full per-atom activation arrays (h1, p1, h2) are exchanged with
    on-chip AllGather collectives between stages.
  - pool2 + dense + readout rows are sharded by MOLECULE, so the
    segment_sum/segment_max readout is purely core-local (matches the
    intended "data-parallel over molecules" sharding for the output).
  - gathers use GPSIMD indirect DMA (one 256/300B descriptor per row).

kernel(**inputs) takes the FULL inputs and returns the FULL [4096, 1]
output; all sharding happens inside.
"""
import contextlib

import numpy as np

import concourse.bass as bass
import concourse.mybir as mybir
import concourse.tile as tile
from concourse import bacc
from concourse.bass_utils import run_bass_kernel_spmd

# ---- static problem shape ----
COUNTS = [1024, 24576, 36864, 36864, 20480, 2048, 512, 256, 128, 64, 64]
OFFS = np.cumsum([0] + COUNTS)
N_ATOMS = int(OFFS[-1])           # 122880
BATCH = 4096
F_IN = 75
HID = 64
MAXDEG = 10
EPS = 1e-3
NCORE = 8
P = 128

FP = mybir.dt.float32
BF = mybir.dt.bfloat16
I32 = mybir.dt.int32


def _pad128(n):
    return (int(n) + P - 1) // P * P


SH_D = [COUNTS[d] // NCORE for d in range(MAXDEG + 1)]   # per-core class size
SH_DP = [_pad128(s) for s in SH_D]
SH_OFF = np.cumsum([0] + SH_DP)
SH_ROWS = int(SH_OFF[-1])
AG_ROWS = SH_ROWS * NCORE

_SH_D_ARR = np.array(SH_D)
_SH_OFF_ARR = np.array(SH_OFF[:-1])


def _global_to_padded(g):
    """global atom id -> row in the padded AllGather activation layout."""
    g = np.asarray(g, np.int64)
    d = np.searchsorted(OFFS, g, side="right") - 1
    p = g - OFFS[d]
    n8 = _SH_D_ARR[d]
    core = p // n8
    loc = p % n8
    return (core * SH_ROWS + _SH_OFF_ARR[d] + loc).astype(np.int32)


def _op_major(cols):
    """[n] index list -> [128, m] tile-op-major layout (op j = col j)."""
    n = len(cols)
    m = n // P
    return np.ascontiguousarray(cols.reshape(m, P).T).astype(np.int32)


class Plan:
    """Host-side index preprocessing (pure rearrangement of input indices)."""

    def __init__(self, membership, adjs):
        self.idx_atoms = {}   # (d, k) -> [NCORE][128, m] global atom ids
        self.idx_pad = {}     # (d, k) -> [NCORE][128, m] padded AG ids
        for d in range(1, MAXDEG + 1):
            adj = np.asarray(adjs[d - 1])
            n8 = SH_D[d]
            for k in range(d):
                ga, pa = [], []
                for c in range(NCORE):
                    sl = np.zeros(SH_DP[d], np.int64)
                    sl[:n8] = adj[c * n8:(c + 1) * n8, k]
                    ga.append(_op_major(sl.astype(np.int32)))
                    pa.append(_op_major(_global_to_padded(sl)))
                self.idx_atoms[(d, k)] = ga
                self.idx_pad[(d, k)] = pa

        # conv1 self rows: per-core [SH_ROWS] global ids (padded tails -> 0)
        self.self_atom_rows = []
        for c in range(NCORE):
            rows = np.zeros(SH_ROWS, np.int64)
            for d in range(MAXDEG + 1):
                o = SH_OFF[d]
                n8 = SH_D[d]
                rows[o:o + n8] = OFFS[d] + c * n8 + np.arange(n8)
            self.self_atom_rows.append(rows)

        # ---- final stage (molecule-owned) ----
        mol_of = np.asarray(membership, np.int64)
        deg_of = np.searchsorted(OFFS, np.arange(N_ATOMS), side="right") - 1
        self.MOLS = BATCH // NCORE
        core_of = mol_of // self.MOLS
        lists = {}
        cnt = np.zeros((NCORE, MAXDEG + 1), np.int64)
        for c in range(NCORE):
            g = np.where(core_of == c)[0]
            dg = deg_of[g]
            for d in range(MAXDEG + 1):
                gl = np.sort(g[dg == d])
                lists[(c, d)] = gl
                cnt[c, d] = len(gl)
        self.F_DP = [_pad128(cnt[:, d].max()) for d in range(MAXDEG + 1)]
        self.F_OFF = np.cumsum([0] + self.F_DP)
        self.F_ROWS = int(self.F_OFF[-1])

        adj_by = {d: np.asarray(adjs[d - 1]) for d in range(1, MAXDEG + 1)}
        self.f_self = []          # [NCORE][128, F_ROWS//128] padded AG ids
        self.f_adj = {}           # (d, k) -> [NCORE][128, m]
        self.f_rows_g = np.zeros((NCORE, self.F_ROWS), np.int64)
        self.f_valid = np.zeros((NCORE, self.F_ROWS), bool)
        for c in range(NCORE):
            srows = np.zeros(self.F_ROWS, np.int64)
            for d in range(MAXDEG + 1):
                gl = lists[(c, d)]
                o = self.F_OFF[d]
                srows[o:o + len(gl)] = gl
                self.f_rows_g[c, o:o + len(gl)] = gl
                self.f_valid[c, o:o + len(gl)] = True
            self.f_self.append(_op_major(_global_to_padded(srows)))
        for d in range(1, MAXDEG + 1):
            for k in range(d):
                arrs = []
                for c in range(NCORE):
                    gl = lists[(c, d)]
                    a = np.zeros(self.F_DP[d], np.int64)
                    if len(gl):
                        a[:len(gl)] = adj_by[d][gl - OFFS[d], k]
                    arrs.append(_op_major(_global_to_padded(a)))
                self.f_adj[(d, k)] = arrs

        # ---- readout slot maps ----
        # z rows per core: F_ROWS real (+2 sentinel tiles appended by kernel:
        # rows [F_ROWS, F_ROWS+128) = -1e30 (max), [F_ROWS+128, ..+256) = 0.
        SENT_MAX = self.F_ROWS
        SENT_SUM = self.F_ROWS + P
        sizes = np.zeros((NCORE, self.MOLS), np.int64)
        atoms_of = {}
        asort = np.argsort(mol_of, kind="stable")
        msorted = mol_of[asort]
        for c in range(NCORE):
            zrow = np.full(N_ATOMS, -1, np.int64)
            vr = self.f_rows_g[c][self.f_valid[c]]
            zrow[vr] = np.where(self.f_valid[c])[0]
            base = c * self.MOLS
            for mi in range(self.MOLS):
                lo = np.searchsorted(msorted, base + mi, side="left")
                hi = np.searchsorted(msorted, base + mi, side="right")
                a = zrow[asort[lo:hi]]
                atoms_of[(c, mi)] = a[a >= 0]
                sizes[c, mi] = len(atoms_of[(c, mi)])
        self.mol_order = np.argsort(-sizes, axis=1, kind="stable")
        max_slots = int(sizes.max())
        # per-slot mol coverage (max over cores -> SPMD-uniform tiling)
        slot_mols = np.zeros(max_slots, np.int64)
        for s in range(max_slots):
            slot_mols[s] = _pad128(max(1, int((sizes > s).sum(axis=1).max())))
        slot_mols[0] = self.MOLS   # every molecule covered by slot 0
        self.slot_mols = slot_mols                    # padded mols per slot
        # build the concatenated slot index stream per core (both maps)
        self.slot_stream_max = []
        self.slot_stream_sum = []
        # per-slot tile structure: [(slot, tile_idx_in_slot, mol_tile)] with
        # mol_tile = which acc tile (0..MOLS_P/128) it accumulates into
        self.slot_tiles = []
        for s in range(max_slots):
            for t in range(slot_mols[s] // P):
                self.slot_tiles.append((s, t))
        def wrap16(flat):
            # dma_gather index layout: idx j at [j % 16, j // 16], the
            # 16-partition block replicated for each of the 8 Q7 cores.
            w = np.zeros((16, len(flat) // 16), np.int16)
            w[np.arange(len(flat)) % 16, np.arange(len(flat)) // 16] = flat
            return np.tile(w, (8, 1))

        for c in range(NCORE):
            sm, ss = [], []
            for s in range(max_slots):
                n = slot_mols[s]
                a_max = np.full(n, SENT_MAX, np.int64)
                a_sum = np.full(n, SENT_SUM, np.int64)
                for pos in range(min(n, self.MOLS)):
                    mi = self.mol_order[c, pos]
                    if sizes[c, mi] > s:
                        a_max[pos] = atoms_of[(c, mi)][s]
                        a_sum[pos] = atoms_of[(c, mi)][s]
                sm.append(a_max)
                ss.append(a_sum)
            self.slot_stream_max.append(wrap16(np.concatenate(sm)))
            self.slot_stream_sum.append(wrap16(np.concatenate(ss)))
        self.n_slot_rows = int(sum(slot_mols))


def build_kernel(plan):
    nc = bacc.Bacc("TRN2", target_bir_lowering=False, debug=False,
                   num_swdge_queues=4)
    MOLS = plan.MOLS          # 512
    MT = MOLS // P            # 4 mol tiles

    # ---------------- I/O ----------------
    atoms = nc.dram_tensor("atoms", [N_ATOMS, F_IN], FP, kind="ExternalInput")
    atoms_self = nc.dram_tensor("atoms_self", [SH_ROWS, F_IN], FP,
                                kind="ExternalInput")
    atoms_rel = nc.dram_tensor("atoms_rel", [SH_ROWS, F_IN], FP,
                               kind="ExternalInput")
    atoms_relT = nc.dram_tensor("atoms_relT", [F_IN, SH_ROWS], BF,
                                kind="ExternalInput")
    atoms_selfT = nc.dram_tensor("atoms_selfT", [F_IN, SH_ROWS], BF,
                                 kind="ExternalInput")
    idxA, idxP1, idxF = {}, {}, {}
    for d in range(1, MAXDEG + 1):
        m = SH_DP[d] // P
        for k in range(d):
            idxA[(d, k)] = nc.dram_tensor(f"ia_{d}_{k}", [P, m], I32,
                                          kind="ExternalInput")
            idxP1[(d, k)] = nc.dram_tensor(f"ip_{d}_{k}", [P, m], I32,
                                           kind="ExternalInput")
        mf = plan.F_DP[d] // P
        for k in range(d):
            idxF[(d, k)] = nc.dram_tensor(f"if_{d}_{k}", [P, mf], I32,
                                          kind="ExternalInput")
    fself = nc.dram_tensor("fself", [P, plan.F_ROWS // P], I32,
                           kind="ExternalInput")
    n_slot_tiles = len(plan.slot_tiles)
    n_slot_rows = n_slot_tiles * P
    slotmax = nc.dram_tensor("slotmax", [P, n_slot_rows // 16], mybir.dt.int16,
                             kind="ExternalInput")
    slotsum = nc.dram_tensor("slotsum", [P, n_slot_rows // 16], mybir.dt.int16,
                             kind="ExternalInput")
    wc1 = nc.dram_tensor("wc1", [2 * MAXDEG + 1, F_IN, HID], BF, kind="ExternalInput")
    wc2 = nc.dram_tensor("wc2", [2 * MAXDEG + 1, HID, HID], FP, kind="ExternalInput")
    bc1r = nc.dram_tensor("bc1r", [P, (2 * MAXDEG + 1) * HID], FP, kind="ExternalInput")
    bc2r = nc.dram_tensor("bc2r", [P, (2 * MAXDEG + 1) * HID], FP, kind="ExternalInput")
    bn1sr = nc.dram_tensor("bn1sr", [P, HID], FP, kind="ExternalInput")
    bn1br = nc.dram_tensor("bn1br", [P, HID], FP, kind="ExternalInput")
    bn3sr = nc.dram_tensor("bn3sr", [P, 128], FP, kind="ExternalInput")
    bn3br = nc.dram_tensor("bn3br", [P, 128], FP, kind="ExternalInput")
    d1wt = nc.dram_tensor("d1wt", [HID, 128], FP, kind="ExternalInput")
    d1br = nc.dram_tensor("d1br", [P, 128], FP, kind="ExternalInput")
    d2wf = nc.dram_tensor("d2wf", [P, 2], FP, kind="ExternalInput")  # prescaled
    d3wr = nc.dram_tensor("d3wr", [15, 1], FP, kind="ExternalInput")
    cbias = nc.dram_tensor("cbias", [P, 1], FP, kind="ExternalInput")
    npad2 = nc.dram_tensor("npad2", [P, 4], FP, kind="ExternalInput")
    xaddT = nc.dram_tensor("xaddT", [15, MOLS], FP, kind="ExternalInput")
    ident = nc.dram_tensor("identity", [P, P], FP, kind="ExternalInput")
    out = nc.dram_tensor("out", [MOLS, 1], FP, kind="ExternalOutput")

    # ---------------- internal DRAM ----------------
    h1_sh = nc.dram_tensor("h1_sh", [SH_ROWS, HID], FP)
    p1_sh = nc.dram_tensor("p1_sh", [SH_ROWS, HID], FP)
    h2_sh = nc.dram_tensor("h2_sh", [SH_ROWS, HID], FP)
    h1 = nc.dram_tensor("h1", [AG_ROWS, HID], FP, addr_space="Shared")
    p1 = nc.dram_tensor("p1", [AG_ROWS, HID], FP, addr_space="Shared")
    h2 = nc.dram_tensor("h2", [AG_ROWS, HID], FP, addr_space="Shared")
    p2b = nc.dram_tensor("p2b", [plan.F_ROWS, HID], FP)
    zbuf = nc.dram_tensor("zbuf", [plan.F_ROWS + 2 * P, 128], FP)

    with tile.TileContext(nc) as tc:
        with contextlib.ExitStack() as ctx:
            cpool = ctx.enter_context(tc.tile_pool(name="consts", bufs=1))
            gpool = ctx.enter_context(tc.tile_pool(name="gather", bufs=12))
            ipool = ctx.enter_context(tc.tile_pool(name="idx", bufs=14))
            tpool = ctx.enter_context(tc.tile_pool(name="tmp", bufs=8))
            pst = ctx.enter_context(tc.tile_pool(name="pst", bufs=2, space="PSUM"))
            psm = ctx.enter_context(tc.tile_pool(name="psm", bufs=2, space="PSUM"))

            idt = cpool.tile([P, P], FP)
            nc.sync.dma_start(out=idt[:], in_=ident[:])
            b1s = cpool.tile([P, HID], FP)
            b1b = cpool.tile([P, HID], FP)
            b3s = cpool.tile([P, 128], FP)
            b3b = cpool.tile([P, 128], FP)
            db1 = cpool.tile([P, 128], FP)
            nc.sync.dma_start(out=b1s[:], in_=bn1sr[:])
            nc.sync.dma_start(out=b1b[:], in_=bn1br[:])
            nc.sync.dma_start(out=b3s[:], in_=bn3sr[:])
            nc.sync.dma_start(out=b3b[:], in_=bn3br[:])
            nc.sync.dma_start(out=db1[:], in_=d1br[:])
            w1t = cpool.tile([F_IN, (2 * MAXDEG + 1) * HID], BF)
            w2t = cpool.tile([HID, (2 * MAXDEG + 1) * HID], FP)
            bb1 = cpool.tile([P, (2 * MAXDEG + 1) * HID], FP)
            bb2 = cpool.tile([P, (2 * MAXDEG + 1) * HID], FP)
            nc.sync.dma_start(out=w1t[:].rearrange("k (w h) -> k w h", w=2 * MAXDEG + 1),
                              in_=wc1.rearrange("w k h -> k w h"))
            nc.sync.dma_start(out=w2t[:].rearrange("k (w h) -> k w h", w=2 * MAXDEG + 1),
                              in_=wc2.rearrange("w k h -> k w h"))
            nc.sync.dma_start(out=bb1[:], in_=bc1r[:])
            nc.sync.dma_start(out=bb2[:], in_=bc2r[:])
            d1w_t = cpool.tile([HID, 128], FP)
            nc.sync.dma_start(out=d1w_t[:], in_=d1wt[:])
            d2w_t = cpool.tile([P, 2], FP)
            nc.sync.dma_start(out=d2w_t[:], in_=d2wf[:])
            d3w_t = cpool.tile([15, 1], FP)
            nc.sync.dma_start(out=d3w_t[:], in_=d3wr[:])
            cb_t = cpool.tile([P, 1], FP)
            nc.sync.dma_start(out=cb_t[:], in_=cbias[:])
            rT1 = cpool.tile([F_IN, SH_ROWS], BF, name="rT1")
            nc.sync.dma_start(out=rT1[:], in_=atoms_relT[:])
            sT1 = cpool.tile([F_IN, SH_ROWS], BF, name="sT1")
            nc.sync.dma_start(out=sT1[:], in_=atoms_selfT[:])
            npd_t = cpool.tile([P, 4], FP)
            nc.sync.dma_start(out=npd_t[:], in_=npad2[:])

            def wslice(wt, widx):
                return wt[:, widx * HID:(widx + 1) * HID]

            def gather_tile(dst_ap, src_dram, idx_tile_col, accum=False):
                nc.gpsimd.indirect_dma_start(
                    out=dst_ap, out_offset=None, in_=src_dram[:],
                    in_offset=bass.IndirectOffsetOnAxis(ap=idx_tile_col, axis=0),
                    compute_op=(mybir.AluOpType.add if accum
                                else mybir.AluOpType.bypass))

            def conv_epilogue(ps_o, bt, bidxs, bn_s, bn_b, dst_slice, width):
                hsb = tpool.tile([P, width], FP, tag="h")
                nc.vector.tensor_tensor(
                    out=hsb[:], in0=ps_o,
                    in1=bt[:, bidxs[0] * width:(bidxs[0] + 1) * width],
                    op=mybir.AluOpType.add)
                if len(bidxs) > 1:
                    nc.vector.tensor_tensor(
                        out=hsb[:], in0=hsb[:],
                        in1=bt[:, bidxs[1] * width:(bidxs[1] + 1) * width],
                        op=mybir.AluOpType.add)
                nc.scalar.activation(hsb[:], hsb[:],
                                     mybir.ActivationFunctionType.Tanh)
                nc.vector.tensor_tensor(out=hsb[:], in0=hsb[:], in1=bn_s[:],
                                        op=mybir.AluOpType.mult)
                nc.vector.tensor_tensor(out=hsb[:], in0=hsb[:], in1=bn_b[:],
                                        op=mybir.AluOpType.add)
                nc.sync.dma_start(out=dst_slice, in_=hsb[:])

            def transpose_to(src_tile, rows, into_pool, tag):
                ps_full = pst.tile([P, P], FP, tag="tr", name="trps")
                ps = ps_full[0:rows, :]
                dstT = into_pool.tile([rows, P], FP, tag="T" + tag)
                nc.tensor.transpose(out=ps[:], in_=src_tile, identity=idt[:])
                nc.vector.tensor_copy(out=dstT[:], in_=ps[:])
                return dstT

            def conv_stage(src_dram, src_f, idx_map, self_reader, wt, bt, dst,
                           sizes, offs, rel_src=None, relT_res=None,
                           selfT_res=None):
                for d in range(MAXDEG + 1):
                    m = sizes[d] // P
                    its = {}
                    if rel_src is None:
                        for k in range(d):
                            it = ipool.tile([P, m], I32, tag="idx")
                            nc.sync.dma_start(out=it[:], in_=idx_map[(d, k)][:])
                            its[k] = it
                    for j in range(m):
                        o = offs[d] + j * P
                        if d >= 1:
                            if relT_res is not None:
                                relT_ap = relT_res[:, o:o + P]
                            else:
                                rel = gpool.tile([P, src_f], FP, tag="rel")
                                if rel_src is not None:
                                    nc.sync.dma_start(out=rel[:],
                                                      in_=rel_src[o:o + P, :])
                                else:
                                    for k in range(d):
                                        gather_tile(rel[:], src_dram,
                                                    its[k][:, j:j + 1],
                                                    accum=(k > 0))
                                relT_ap = transpose_to(rel[:], src_f,
                                                       tpool, "r")[:]
                        if selfT_res is not None:
                            slfT_ap = selfT_res[:, o:o + P]
                        else:
                            slf = gpool.tile([P, src_f], FP, tag="slf")
                            self_reader(d, j, slf)
                            slfT_ap = transpose_to(slf[:], src_f,
                                                   tpool, "s")[:]
                        ps_o = psm.tile([P, HID], FP, tag="mm")
                        if d >= 1:
                            nc.tensor.matmul(out=ps_o[:], lhsT=relT_ap,
                                             rhs=wslice(wt, 2 * (d - 1)),
                                             start=True, stop=False)
                            nc.tensor.matmul(out=ps_o[:], lhsT=slfT_ap,
                                             rhs=wslice(wt, 2 * d - 1),
                                             start=False, stop=True)
                            bidxs = [2 * (d - 1), 2 * d - 1]
                        else:
                            nc.tensor.matmul(out=ps_o[:], lhsT=slfT_ap,
                                             rhs=wslice(wt, 2 * MAXDEG),
                                             start=True, stop=True)
                            bidxs = [2 * MAXDEG]
                        conv_epilogue(ps_o[:], bt, bidxs, b1s, b1b,
                                      dst[offs[d] + j * P:offs[d] + (j + 1) * P, :],
                                      HID)

            def pool_stage(src_dram, idx_map, self_reader, dst, sizes, offs):
                for d in range(MAXDEG + 1):
                    m = sizes[d] // P
                    its = {}
                    for k in range(d):
                        it = ipool.tile([P, m], I32, tag="idx")
                        nc.sync.dma_start(out=it[:], in_=idx_map[(d, k)][:])
                        its[k] = it
                    for j in range(m):
                        acc = gpool.tile([P, HID], FP, tag="pacc")
                        self_reader(d, j, acc)
                        for k in range(d):
                            g = gpool.tile([P, HID], FP, tag="pg")
                            gather_tile(g[:], src_dram, its[k][:, j:j + 1])
                            nc.vector.tensor_tensor(out=acc[:], in0=acc[:],
                                                    in1=g[:],
                                                    op=mybir.AluOpType.max)
                        nc.sync.dma_start(
                            out=dst[offs[d] + j * P:offs[d] + (j + 1) * P, :],
                            in_=acc[:])

            def allgather(src_sh, full):
                nc.gpsimd.collective_compute(
                    "AllGather", mybir.AluOpType.bypass,
                    replica_groups=[list(range(NCORE))],
                    ins=[src_sh[:]], outs=[full[:]])

            # ---- conv1 ----
            def self_conv1(d, j, dst_tile):
                o = SH_OFF[d] + j * P
                nc.sync.dma_start(out=dst_tile[:], in_=atoms_self[o:o + P, :])

            conv_stage(atoms, F_IN, idxA, self_conv1, w1t, bb1, h1_sh,
                       SH_DP, SH_OFF, relT_res=rT1, selfT_res=sT1)
            allgather(h1_sh, h1)

            # ---- pool1 ---- (self rows are the local shard block)
            def self_local(src):
                def rd(d, j, dst_tile):
                    o = SH_OFF[d] + j * P
                    nc.sync.dma_start(out=dst_tile[:], in_=src[o:o + P, :])
                return rd

            pool_stage(h1, idxP1, self_local(h1_sh), p1_sh, SH_DP, SH_OFF)
            allgather(p1_sh, p1)

            # ---- conv2 ----
            conv_stage(p1, HID, idxP1, self_local(p1_sh), w2t, bb2, h2_sh,
                       SH_DP, SH_OFF)
            allgather(h2_sh, h2)

            # ---- pool2 (molecule-owned rows) ----
            fs_t = ipool.tile([P, plan.F_ROWS // P], I32, tag="fself")
            nc.sync.dma_start(out=fs_t[:], in_=fself[:])

            def self_f(d, j, dst_tile):
                col = (plan.F_OFF[d] // P) + j
                gather_tile(dst_tile[:], h2, fs_t[:, col:col + 1])

            pool_stage(h2, idxF, self_f, p2b, plan.F_DP, plan.F_OFF)

            # ---- dense d1 + tanh + bn3 -> zbuf ----
            for j in range(plan.F_ROWS // P):
                pt = gpool.tile([P, HID], FP, tag="pacc")
                nc.sync.dma_start(out=pt[:], in_=p2b[j * P:(j + 1) * P, :])
                ptT = transpose_to(pt[:], HID, tpool, "z")
                ps_o = psm.tile([P, 128], FP, tag="mmz")
                nc.tensor.matmul(out=ps_o[:], lhsT=ptT[:], rhs=d1w_t[:],
                                 start=True, stop=True)
                zt = tpool.tile([P, 128], FP, tag="zt")
                nc.vector.tensor_tensor(out=zt[:], in0=ps_o[:], in1=db1[:],
                                        op=mybir.AluOpType.add)
                nc.scalar.activation(zt[:], zt[:],
                                     mybir.ActivationFunctionType.Tanh)
                nc.vector.tensor_tensor(out=zt[:], in0=zt[:], in1=b3s[:],
                                        op=mybir.AluOpType.mult)
                nc.vector.tensor_tensor(out=zt[:], in0=zt[:], in1=b3b[:],
                                        op=mybir.AluOpType.add)
                nc.sync.dma_start(out=zbuf[j * P:(j + 1) * P, :], in_=zt[:])
            # sentinel tiles
            sent = tpool.tile([P, 128], FP, tag="zt")
            nc.vector.memset(sent[:], -2.0)
            nc.sync.dma_start(out=zbuf[plan.F_ROWS:plan.F_ROWS + P, :], in_=sent[:])
            sent2 = tpool.tile([P, 128], FP, tag="zt")
            nc.vector.memset(sent2[:], 0.0)
            nc.sync.dma_start(out=zbuf[plan.F_ROWS + P:plan.F_ROWS + 2 * P, :],
                              in_=sent2[:])

            # ---- readout: slot-gather accumulate ----
            smx_t = ipool.tile([P, n_slot_rows // 16], mybir.dt.int16, tag="smx")
            nc.sync.dma_start(out=smx_t[:], in_=slotmax[:])
            accs = [cpool.tile([P, 128], FP, name=f"accs{t}") for t in range(MT)]
            accm = [cpool.tile([P, 128], FP, name=f"accm{t}") for t in range(MT)]
            first_s = [True] * MT
            first_m = [True] * MT
            CH_RO = 4096            # slot rows per dma_gather op
            TPC = CH_RO // P        # tiles per chunk
            qn = [0]
            for base in range(0, n_slot_tiles, TPC):
                ntile = min(TPC, n_slot_tiles - base)
                nidx = ntile * P
                gt = gpool.tile([P, TPC * 128], FP, tag="rog", bufs=2)
                nc.gpsimd.dma_gather(
                    out_ap=gt[:, :ntile * 128].rearrange(
                        "p (c f) -> p c f", c=ntile),
                    in_ap=zbuf[:],
                    idxs_ap=smx_t[:, (base * P) // 16:(base * P + nidx) // 16],
                    num_idxs=nidx, num_idxs_reg=nidx, elem_size=128,
                    single_packet=False, queue_num=qn[0] % 4,
                )
                qn[0] += 1
                for ci in range(ntile):
                    s, t = plan.slot_tiles[base + ci]
                    sl = gt[:, ci * 128:(ci + 1) * 128]
                    for acc, first, op in (
                            (accm, first_m, mybir.AluOpType.max),
                            (accs, first_s, mybir.AluOpType.add)):
                        if first[t]:
                            nc.vector.tensor_copy(out=acc[t][:], in_=sl)
                            first[t] = False
                        else:
                            nc.vector.tensor_tensor(out=acc[t][:],
                                                    in0=acc[t][:],
                                                    in1=sl, op=op)

            # ---- final: out = tanh(s|m) @ d2w' + xadd @ d3w[1:] + cb ----
            xa_t = cpool.tile([15, MOLS], FP)
            nc.sync.dma_start(out=xa_t[:], in_=xaddT[:])
            for t in range(MT):
                nc.vector.tensor_scalar(
                    out=accs[t][:], in0=accs[t][:],
                    scalar1=npd_t[:, t:t + 1], scalar2=None,
                    op0=mybir.AluOpType.add)
                nc.scalar.activation(accs[t][:], accs[t][:],
                                     mybir.ActivationFunctionType.Tanh)
                nc.scalar.activation(accm[t][:], accm[t][:],
                                     mybir.ActivationFunctionType.Tanh)
                sT = transpose_to(accs[t][:], P, tpool, "f")
                mT = transpose_to(accm[t][:], P, tpool, "f")
                ps_f = psm.tile([P, 1], FP, tag="fin")
                nc.tensor.matmul(out=ps_f[:], lhsT=sT[:], rhs=d2w_t[:, 0:1],
                                 start=True, stop=False)
                nc.tensor.matmul(out=ps_f[:], lhsT=mT[:], rhs=d2w_t[:, 1:2],
                                 start=False, stop=False)
                nc.tensor.matmul(out=ps_f[:], lhsT=xa_t[:, t * P:(t + 1) * P],
                                 rhs=d3w_t[:], start=False, stop=True)
                ot = tpool.tile([P, 1], FP, tag="ot")
                nc.vector.tensor_tensor(out=ot[:], in0=ps_f[:], in1=cb_t[:],
                                        op=mybir.AluOpType.add)
                nc.sync.dma_start(out=out[t * P:(t + 1) * P, :], in_=ot[:])

    nc.compile()
    return nc


_CACHE = {}


def kernel(**inputs):
    atoms = np.asarray(inputs["atoms"], np.float32)
    membership = np.asarray(inputs["membership"], np.int32)
    adjs = [np.asarray(inputs[f"adj{d}"], np.int32) for d in range(1, MAXDEG + 1)]

    key = "k"
    if key not in _CACHE:
        plan = Plan(membership, adjs)
        nc = build_kernel(plan)
        _CACHE[key] = (plan, nc)
    plan, nc = _CACHE[key]

    # ---- parameter preprocessing (host, O(param size)) ----
    gc1_W = np.asarray(inputs["gc1_W"], np.float32)
    gc1_b = np.asarray(inputs["gc1_b"], np.float32)
    gc2_W = np.asarray(inputs["gc2_W"], np.float32)
    gc2_b = np.asarray(inputs["gc2_b"], np.float32)
    bn1_s = (np.asarray(inputs["bn1_gamma"], np.float32)
             / np.sqrt(np.asarray(inputs["bn1_var"], np.float32) + EPS))
    bn1_b = (np.asarray(inputs["bn1_beta"], np.float32)
             - np.asarray(inputs["bn1_mean"], np.float32) * bn1_s)
    bn3_s = (np.asarray(inputs["bn3_gamma"], np.float32)
             / np.sqrt(np.asarray(inputs["bn3_var"], np.float32) + EPS))
    bn3_b = (np.asarray(inputs["bn3_beta"], np.float32)
             - np.asarray(inputs["bn3_mean"], np.float32) * bn3_s)
    d1_W = np.asarray(inputs["d1_W"], np.float32)
    d1_b = np.asarray(inputs["d1_b"], np.float32)
    d2_W = np.asarray(inputs["d2_W"], np.float32)
    d2_b = np.asarray(inputs["d2_b"], np.float32)
    d3_W = np.asarray(inputs["d3_W"], np.float32)
    d3_b = np.asarray(inputs["d3_b"], np.float32)
    x_add = np.asarray(inputs["x_add"], np.float32)

    d2w_scaled = d2_W * d3_W[0, 0]                       # [256, 1]
    cb = float(d2_b[0] * d3_W[0, 0] + d3_b[0])

    def rep(v, w):
        return np.tile(np.asarray(v, np.float32)[None, :], (P, 1)).reshape(P, w)

    common = {
        "atoms": atoms,
        "wc1": __import__("ml_dtypes").bfloat16(gc1_W), "wc2": gc2_W,
        "bc1r": np.tile(gc1_b.reshape(1, -1), (P, 1)),
        "bc2r": np.tile(gc2_b.reshape(1, -1), (P, 1)),
        "bn1sr": rep(bn1_s, HID), "bn1br": rep(bn1_b, HID),
        "bn3sr": rep(bn3_s, 128), "bn3br": rep(bn3_b, 128),
        "d1wt": d1_W, "d1br": rep(d1_b, 128),
        "d2wf": np.ascontiguousarray(d2w_scaled.reshape(2, P).T),
        "d3wr": d3_W[1:16],
        "cbias": np.full((P, 1), cb, np.float32),
        "identity": np.eye(P, dtype=np.float32),
    }
    in_maps = []
    for c in range(NCORE):
        m = dict(common)
        m["atoms_self"] = np.ascontiguousarray(atoms[plan.self_atom_rows[c]])
        rel1 = np.zeros((SH_ROWS, F_IN), np.float32)
        for d in range(1, MAXDEG + 1):
            n8 = SH_D[d]
            blk = atoms[adjs[d - 1][c * n8:(c + 1) * n8]].sum(axis=1)
            rel1[SH_OFF[d]:SH_OFF[d] + n8] = blk
        m["atoms_rel"] = rel1
        import ml_dtypes
        m["atoms_relT"] = np.ascontiguousarray(rel1.T).astype(ml_dtypes.bfloat16)
        m["atoms_selfT"] = np.ascontiguousarray(
            atoms[plan.self_atom_rows[c]].T).astype(ml_dtypes.bfloat16)
        for d in range(1, MAXDEG + 1):
            for k in range(d):
                m[f"ia_{d}_{k}"] = plan.idx_atoms[(d, k)][c]
                m[f"ip_{d}_{k}"] = plan.idx_pad[(d, k)][c]
                m[f"if_{d}_{k}"] = plan.f_adj[(d, k)][c]
        m["fself"] = plan.f_self[c]
        m["slotmax"] = plan.slot_stream_max[c]
        m["slotsum"] = plan.slot_stream_sum[c]
        cover = np.array([(plan.slot_mols > pos).sum()
                          for pos in range(plan.MOLS)], np.int64)
        counts = np.bincount(np.asarray(membership, np.int64),
                             minlength=BATCH)
        sz = counts[c * plan.MOLS + plan.mol_order[c]]
        npad_pos = cover - sz
        assert npad_pos.min() >= 0
        m["npad2"] = np.ascontiguousarray(
            (2.0 * npad_pos.reshape(4, P).T).astype(np.float32))
        xa = x_add[c * plan.MOLS:(c + 1) * plan.MOLS][plan.mol_order[c]]
        m["xaddT"] = np.ascontiguousarray(xa.T)          # [15, MOLS]
        in_maps.append(m)

    res = run_bass_kernel_spmd(nc, in_maps, core_ids=list(range(NCORE)))
    kernel._last_results = res

    outp = np.zeros((BATCH, 1), np.float32)
    for c in range(NCORE):
        o = np.asarray(res.results[c]["out"])            # sorted-mol order
        inv = np.empty(plan.MOLS, np.int64)
        inv[plan.mol_order[c]] = np.arange(plan.MOLS)
        outp[c * plan.MOLS:(c + 1) * plan.MOLS] = o[inv]
    return outp

